# revision 39
# baseline (speedup 1.0000x reference)
"""CapsuleLayer dynamic-routing kernel for 8 Trainium2 NeuronCores.

Problem: x [64,2048,16], route_weights [32,2048,16,32] ->
  3-iteration routing -> out [32,64,1,1,32] (fp32).

Sharding: capsules (C=32) split 4-per-core across 8 cores.

All data fp16 (measured ~7e-3 rel err, tol 2e-2); logits fp32 at iter 2.
Per core / iteration (engines balanced ~90% in the V phase):
  phase A : s1 = (1/R) sum_(j,r) x W     PE on resident wcat (fp16)
  V pass  : psV[cb,512] = blockdiag(outT)^T @ wt-chunk  (PE, streamed wt)
            ACT evacuates psV -> vs fp16; DVE: xv = x2d * vs (2x mode);
            j-reduce split DVE (tensor_reduce) / GpSimd (pairwise tree)
  softmax : max (DVE), exp+Z (ACT, fp16 out), [128,128] transposes (PE)
  s pass  : xe[rj,(t,c,b)] = xt2 * eT (DVE, t-halves for early start)
            psS[co,(c,b)] += wcat-chunk^T @ xe  (PE, N=256, one PSUM bank;
            only diagonal c-blocks used) ; transpose + 1/Z fold ; squash
"""
import os
import numpy as np

C, B, R, CIN, OUT = 32, 64, 2048, 16, 32
NCORES = 8
CLOC = C // NCORES          # 4 capsules per core
RJ = R * CIN                # 32768
NK = RJ // 128              # 256 chunks of 128 along (j,r)
NV = RJ // 512              # 64 chunks of 512 along (r,j) for V pass

_CACHE = {}


def _build_program():
    from contextlib import ExitStack
    import concourse.bass as bass
    import concourse.bacc as bacc
    import concourse.tile as tile
    from concourse import mybir

    f32 = mybir.dt.float32
    f16 = mybir.dt.float16
    AL = mybir.AluOpType
    AF = mybir.ActivationFunctionType
    AX = mybir.AxisListType

    nc = bacc.Bacc(None, target_bir_lowering=False,
                   detect_race_conditions=not bool(int(os.environ.get("CAPS_NO_RACE", "0"))))
    n_loops = int(os.environ.get("CAPS_LOOPS", "1"))

    # ---- DRAM I/O (all host-swizzled to [partition, free] layouts) ----
    wcat = nc.dram_tensor("wcat", [128, NK * 128], f16, kind="ExternalInput")  # [p,(k,(c,o))]
    xt2 = nc.dram_tensor("xt2", [128, NK * B], f16, kind="ExternalInput")      # [p,(k,b)]
    wt = nc.dram_tensor("wt", [128, RJ], f16, kind="ExternalInput")            # [(c,o),(r,j)]
    x2d = nc.dram_tensor("x2d", [128, RJ], f16, kind="ExternalInput")          # [(2,b),(r,j)]
    ident = nc.dram_tensor("ident", [128, 128], f16, kind="ExternalInput")
    out3 = nc.dram_tensor("out3", [B, 128], f32, kind="ExternalOutput")        # [b,(c,o)]
    debug = bool(int(os.environ.get("CAPS_DEBUG", "0")))
    if debug:
        dbg_o1 = nc.dram_tensor("dbg_o1", [B, 128], f32, kind="ExternalOutput")
        dbg_lp = nc.dram_tensor("dbg_lp", [128, R], f32, kind="ExternalOutput")
        dbg_p2 = nc.dram_tensor("dbg_p2", [128, CLOC * (R // 128) * B], f16,
                                kind="ExternalOutput")
        dbg_s2 = nc.dram_tensor("dbg_s2", [B, 128], f32, kind="ExternalOutput")

    with tile.TileContext(nc) as tc, ExitStack() as ctx:
        const = ctx.enter_context(tc.tile_pool(name="const", bufs=1))
        small = ctx.enter_context(tc.tile_pool(name="small", bufs=2))
        wt_p = ctx.enter_context(tc.tile_pool(name="wtp", bufs=3))
        x2_p = ctx.enter_context(tc.tile_pool(name="x2p", bufs=2))
        vs_p = ctx.enter_context(tc.tile_pool(name="vsp", bufs=2))
        xe_p = ctx.enter_context(tc.tile_pool(name="xep", bufs=2))
        eP_p = ctx.enter_context(tc.tile_pool(name="ePp", bufs=2))
        tree_p = ctx.enter_context(tc.tile_pool(name="tree", bufs=1))
        psS_p = ctx.enter_context(tc.tile_pool(name="psS", bufs=1, space="PSUM"))
        psV_p = ctx.enter_context(tc.tile_pool(name="psV", bufs=4, space="PSUM"))
        psT_p = ctx.enter_context(tc.tile_pool(name="psT", bufs=2, space="PSUM"))

        idn = const.tile([128, 128], f16, tag="ident", name="idn")
        nc.sync.dma_start(out=idn, in_=ident[:])

        # resident W in [(j,r)-chunk, (c,o)] layout, fp16  (8.39 MB)
        wcat_sb = const.tile([128, NK, 128], f16, tag="wcat", name="wcat_sb")
        # resident x^T [(j,r)-chunk, b] fp16  (4.19 MB)
        xt2_sb = const.tile([128, NK, B], f16, tag="xt2sb", name="xt2_sb")
        NSPLIT = 16
        for i in range(NSPLIT):
            s0, s1 = NK // NSPLIT * i, NK // NSPLIT * (i + 1)
            nc.sync.dma_start(
                out=wcat_sb[:, s0:s1, :].rearrange("p k m -> p (k m)"),
                in_=wcat[:, 128 * s0:128 * s1])
            nc.sync.dma_start(
                out=xt2_sb[:, s0:s1, :].rearrange("p k b -> p (k b)"),
                in_=xt2[:, B * s0:B * s1])

        # logits per capsule-pair [(2c,b)=128, r=2048]
        # iter1 logits in fp16 (delta ~ +-10, err ~4e-3); iter2 in fp32
        lP16 = [const.tile([128, R], f16, tag=f"l16{p}", name=f"lP16{p}")
                for p in range(2)]
        # delta accumulator blocks (iteration 2)
        dblk = [const.tile([128, R], f32, tag=f"d{p}", name=f"dblk{p}") for p in range(2)]
        # transposed probs [128=r%128, (rb=16, c=4, b=64)] fp16 (flat for xe)
        p2T = const.tile([128, R // 128, CLOC, B], f16, tag="p2T", name="p2T")
        # blockdiag outT operand, capsule pair pr at partitions [64pr:64pr+64]
        oT2 = const.tile([128, 128], f16, tag="o2", name="oT2")
        nc.vector.memset(oT2[:], 0.0)

        def squash(u_bT, scale_pow, final=False):
            """u_bT [64,(4c,32o)] fp32: s=u*scale_pow; out=s*sqrt(n2)/(n2+1).

            Emits o_i (fp32 [B,128]) and fills oT2 blockdiag tiles (fp16)."""
            sq = small.tile([B, 128], f32, tag="sq", name="sq")
            nc.vector.scalar_tensor_tensor(
                out=sq, in0=u_bT, scalar=float(scale_pow * scale_pow),
                in1=u_bT, op0=AL.mult, op1=AL.mult)
            n2 = small.tile([B, CLOC], f32, tag="n2", name="n2")
            nc.vector.tensor_reduce(
                out=n2, in_=sq[:].rearrange("b (c o) -> b c o", c=CLOC),
                axis=AX.X, op=AL.add)
            rt = small.tile([B, CLOC], f32, tag="rt", name="rt")
            nc.scalar.activation(out=rt, in_=n2, func=AF.Sqrt)
            dn = small.tile([B, CLOC], f32, tag="dn", name="dn")
            nc.vector.tensor_scalar_add(out=dn, in0=n2, scalar1=1.0)
            rc = small.tile([B, CLOC], f32, tag="rc", name="rc")
            nc.vector.reciprocal(out=rc, in_=dn)
            f = small.tile([B, CLOC], f32, tag="f", name="f")
            nc.vector.tensor_mul(out=f, in0=rt, in1=rc)
            f2 = small.tile([B, CLOC], f32, tag="f2", name="f2")
            nc.vector.tensor_scalar_mul(out=f2, in0=f, scalar1=float(scale_pow))
            o_i = small.tile([B, 128], f32, tag="oi", name="oi")
            f2b = bass.AP(tensor=f2[:].tensor, offset=f2[:].offset,
                          ap=[f2[:].ap[0], f2[:].ap[1], [0, OUT]])
            nc.vector.tensor_tensor(
                out=o_i[:].rearrange("b (c o) -> b c o", c=CLOC),
                in0=u_bT[:].rearrange("b (c o) -> b c o", c=CLOC),
                in1=f2b, op=AL.mult)
            if final:
                return o_i
            o16 = small.tile([B, 128], f16, tag="o16", name="o16")
            nc.vector.tensor_copy(out=o16, in_=o_i)
            psOT = psT_p.tile([128, 128], f16, tag="psT", name="psOT")[:, 0:B]
            nc.tensor.transpose(psOT, o16, idn[0:B, 0:B])
            for p in range(2):
                nc.scalar.copy(out=oT2[64 * p:64 * p + 32, 0:64],
                               in_=psOT[64 * p:64 * p + 32, :])
                nc.scalar.copy(out=oT2[64 * p + 32:64 * p + 64, 64:128],
                               in_=psOT[64 * p + 32:64 * p + 64, :])
            return o_i

        xt2_flat = xt2_sb[:].rearrange("p k b -> p (k b)")
        p2T_flat = p2T[:].rearrange("p r c b -> p (r c b)")

        for _loop in range(n_loops):
            # ---------- Phase A: s1 = (1/R) sum_(j,r) x W ----------
            psA = psV_p.tile([128, 512], f32, tag="psV", name="psA")[:, 0:B]
            for k in range(NK):
                nc.tensor.matmul(psA, wcat_sb[:, k, :], xt2_sb[:, k, :],
                                 start=(k == 0), stop=(k == NK - 1))
            sA = small.tile([128, B], f16, tag="sA", name="sA")
            nc.scalar.copy(out=sA, in_=psA)
            psAT = psT_p.tile([128, 128], f16, tag="psT", name="psAT")[0:B, :]
            nc.tensor.transpose(psAT, sA, idn)
            uT = small.tile([B, 128], f32, tag="uT", name="uT")
            nc.scalar.copy(out=uT, in_=psAT)
            o1 = squash(uT, 1.0 / R)
            if debug:
                nc.sync.dma_start(out=dbg_o1[:], in_=o1)

            # ---------- Two routing boundaries ----------
            for it in (1, 2):
                # --- V pass: streamed wt/x2d quads of 2048 cols ---
                for g in range(16):
                    wtg = wt_p.tile([128, 2048], f16, tag="wtk", name="wtk")
                    nc.sync.dma_start(out=wtg, in_=wt[:, 2048 * g:2048 * (g + 1)])
                    x2g = x2_p.tile([128, 2048], f16, tag="x2k", name="x2k")
                    nc.sync.dma_start(out=x2g, in_=x2d[:, 2048 * g:2048 * (g + 1)])
                    for pr in range(2):
                        vs4 = vs_p.tile([128, 2048], f16, tag="vs", name="vs")
                        for q in range(4):
                            psV = psV_p.tile([128, 512], f32, tag="psV", name="psV")
                            nc.tensor.matmul(
                                psV, oT2[64 * pr:64 * (pr + 1), :],
                                wtg[64 * pr:64 * (pr + 1), 512 * q:512 * (q + 1)],
                                start=True, stop=True)
                            nc.scalar.copy(out=vs4[:, 512 * q:512 * (q + 1)],
                                           in_=psV)
                        xv4 = vs_p.tile([128, 2048], f16, tag="xv", name="xv")
                        nc.vector.tensor_mul(out=xv4, in0=x2g, in1=vs4)
                        dst = lP16[pr] if it == 1 else dblk[pr]
                        dsl = dst[:, 128 * g:128 * (g + 1)]
                        xv3 = xv4[:].rearrange("p (r j) -> p r j", j=CIN)
                        with nc.allow_low_precision(
                                reason="iter1 delta ~ +-10, fp16 err 4e-3 ok"):
                            if pr == 0 or g in (5, 11):
                                # DVE grouped reduce
                                nc.vector.tensor_reduce(
                                    out=dsl, in_=xv3, axis=AX.X, op=AL.add)
                            else:
                                # GpSimd pairwise tree (fp32 intermediates)
                                t1 = tree_p.tile([128, 128, 8], f32, tag="t1",
                                                 name="t1")
                                nc.gpsimd.tensor_tensor(
                                    out=t1, in0=xv3[:, :, 0:8],
                                    in1=xv3[:, :, 8:16], op=AL.add)
                                t2 = tree_p.tile([128, 128, 4], f32, tag="t2",
                                                 name="t2")
                                nc.gpsimd.tensor_tensor(
                                    out=t2, in0=t1[:, :, 0:4], in1=t1[:, :, 4:8],
                                    op=AL.add)
                                t3 = tree_p.tile([128, 128, 2], f32, tag="t3",
                                                 name="t3")
                                nc.gpsimd.tensor_tensor(
                                    out=t3, in0=t2[:, :, 0:2], in1=t2[:, :, 2:4],
                                    op=AL.add)
                                nc.gpsimd.tensor_tensor(
                                    out=dsl.rearrange("p (r o) -> p r o", o=1),
                                    in0=t3[:, :, 0:1], in1=t3[:, :, 1:2],
                                    op=AL.add)
                if it == 2:
                    for pr in range(2):
                        nc.vector.tensor_add(out=dblk[pr], in0=lP16[pr],
                                             in1=dblk[pr])
                if debug and it == 1:
                    dbgl = small.tile([128, R], f32, tag="dbgl", name="dbgl")
                    nc.vector.tensor_copy(out=dbgl, in_=lP16[0])
                    nc.sync.dma_start(out=dbg_lp[:], in_=dbgl)

                # --- softmax pieces (unnormalized e + 1/Z) ---
                rzq = []
                for pr in range(2):
                    lsrc = lP16[pr] if it == 1 else dblk[pr]
                    m = small.tile([128, 1], f32, tag="m", name="m")
                    nc.vector.tensor_reduce(out=m, in_=lsrc, axis=AX.X, op=AL.max)
                    mneg = small.tile([128, 1], f32, tag="mneg", name="mneg")
                    nc.vector.tensor_scalar_mul(out=mneg, in0=m, scalar1=-1.0)
                    eP = eP_p.tile([128, R], f16, tag="eP", name="eP")
                    Z = small.tile([128, 1], f32, tag="Z", name="Z")
                    nc.scalar.activation(out=eP, in_=lsrc, func=AF.Exp,
                                         bias=mneg[:, 0:1], scale=1.0, accum_out=Z)
                    for ce in range(2):
                        rz = small.tile([B, 1], f32, tag=f"rz{2*pr+ce}", name="rz")
                        nc.vector.reciprocal(out=rz, in_=Z[64 * ce:64 * (ce + 1), 0:1])
                        rzq.append(rz)
                    for rb in range(R // 128):
                        psT2 = psT_p.tile([128, 128], f16, tag="psT", name="psT2")
                        nc.tensor.transpose(
                            psT2, eP[:, 128 * rb:128 * (rb + 1)], idn)
                        nc.vector.tensor_copy(
                            out=p2T[:, rb, 2 * pr:2 * pr + 2, :].rearrange(
                                "p e b -> p (e b)"),
                            in_=psT2)

                # --- xe + s matmuls: psS[co, (c,b)] = wcat^T @ xe4 ---
                # xe in [128, (t8, c4, b)] layout; only diagonal (c,c) psS
                # blocks are used. One matmul per 128-chunk (N=256).
                psS = psS_p.tile([128, CLOC * B], f32, tag="psS", name="psS")
                for j in range(CIN):
                    xe = xe_p.tile([128, 16, CLOC, B], f16, tag="xe",
                                   name="xe")
                    halves = ((0, 8), (8, 16)) if j <= 1 else ((0, 16),)
                    for (t0, t1) in halves:
                        nt = t1 - t0
                        c0 = B * (16 * j + t0)
                        x_sl = xt2_flat[:, c0:c0 + nt * B]
                        x_bc = bass.AP(tensor=x_sl.tensor, offset=x_sl.offset,
                                       ap=[x_sl.ap[0], [B, nt], [0, CLOC],
                                           [1, B]])
                        nc.vector.tensor_tensor(
                            out=xe[:, t0:t1, :, :], in0=x_bc,
                            in1=p2T[:, t0:t1, :, :],
                            op=AL.mult)
                        for t in range(t0, t1):
                            k = 16 * j + t
                            nc.tensor.matmul(
                                psS, wcat_sb[:, k, :],
                                xe[:, t, :, :],
                                start=(k == 0), stop=(k == NK - 1))
                sSt = small.tile([128, CLOC * B], f16, tag="sSt", name="sSt")
                nc.scalar.copy(out=sSt, in_=psS)
                sS = small.tile([B, 128], f32, tag="sS", name="sS")
                for c4 in range(CLOC):
                    psX = psT_p.tile([128, 128], f16, tag="psT",
                                     name="psX")[0:B, 0:32]
                    nc.tensor.transpose(
                        psX, sSt[32 * c4:32 * (c4 + 1), B * c4:B * (c4 + 1)],
                        idn[32 * c4:32 * c4 + 32, 32 * c4:32 * c4 + 32],
                        tile_position=(32 * c4, 0))
                    nc.scalar.activation(out=sS[:, 32 * c4:32 * (c4 + 1)],
                                         in_=psX,
                                         func=AF.Copy, bias=0.0,
                                         scale=rzq[c4][:, 0:1])
                if debug and it == 1:
                    nc.sync.dma_start(out=dbg_p2[:], in_=p2T_flat)
                    nc.sync.dma_start(out=dbg_s2[:], in_=sS)
                o_i = squash(sS, 1.0, final=(it == 2))

            nc.sync.dma_start(out=out3[:], in_=o_i)

    nc.finalize()
    return nc


def _get_program():
    if "nc" not in _CACHE:
        _CACHE["nc"] = _build_program()
    return _CACHE["nc"]


def make_in_maps(x, route_weights):
    x = np.ascontiguousarray(x, dtype=np.float32)
    W = np.ascontiguousarray(route_weights, dtype=np.float32)
    # x^T [(j,r), b] -> [p=128, (k, b)] fp16
    xt2f = x.transpose(2, 1, 0).reshape(NK, 128, B).transpose(1, 0, 2)
    xt2_h = np.ascontiguousarray(xt2f.reshape(128, NK * B), dtype=np.float16)
    # x natural [b, (r,j)] duplicated -> [128, RJ] fp16
    xnat = x.reshape(B, RJ)
    x2d_h = np.ascontiguousarray(
        np.concatenate([xnat, xnat], axis=0), dtype=np.float16)
    ident = np.eye(128, dtype=np.float16)
    in_maps = []
    for core in range(NCORES):
        wc = W[CLOC * core:CLOC * (core + 1)]                  # [4,R,J,O]
        # [(j,r), (c,o)] -> [p, (k, co)] fp16
        wcf = wc.transpose(2, 1, 0, 3).reshape(NK, 128, 128).transpose(1, 0, 2)
        wcat_h = np.ascontiguousarray(wcf.reshape(128, NK * 128), dtype=np.float16)
        # [(c,o), (r,j)] fp16
        wt_h = np.ascontiguousarray(
            wc.transpose(0, 3, 1, 2).reshape(128, RJ), dtype=np.float16)
        in_maps.append({"wcat": wcat_h, "xt2": xt2_h, "wt": wt_h,
                        "x2d": x2d_h, "ident": ident})
    return in_maps


def kernel(x, route_weights):
    from concourse.bass_utils import run_bass_kernel_spmd

    in_maps = make_in_maps(x, route_weights)
    nc = _get_program()
    res = run_bass_kernel_spmd(nc, in_maps, core_ids=list(range(NCORES)))
    if os.environ.get("CAPS_RESULT_STASH"):
        _CACHE["last_result"] = res

    out = np.empty((C, B, 1, 1, OUT), dtype=np.float32)
    for core in range(NCORES):
        o = res.results[core]["out3"].reshape(B, CLOC, OUT).transpose(1, 0, 2)
        out[CLOC * core:CLOC * (core + 1), :, 0, 0, :] = o
    return out


# revision 40
# speedup vs baseline: 1.0263x; 1.0263x over previous
"""CapsuleLayer dynamic-routing kernel for 8 Trainium2 NeuronCores.

Problem: x [64,2048,16], route_weights [32,2048,16,32] ->
  3-iteration routing -> out [32,64,1,1,32] (fp32).

Sharding: capsules (C=32) split 4-per-core across 8 cores.

All data fp16 (measured ~7e-3 rel err, tol 2e-2); logits fp32 at iter 2.
Per core / iteration (engines balanced ~90% in the V phase):
  phase A : s1 = (1/R) sum_(j,r) x W     PE on resident wcat (fp16)
  V pass  : psV[cb,512] = blockdiag(outT)^T @ wt-chunk  (PE, streamed wt)
            ACT evacuates psV -> vs fp16; DVE: xv = x2d * vs (2x mode);
            j-reduce split DVE (tensor_reduce) / GpSimd (pairwise tree)
  softmax : max (DVE), exp+Z (ACT, fp16 out), [128,128] transposes (PE)
  s pass  : xe[rj,(t,c,b)] = xt2 * eT (DVE, t-halves for early start)
            psS[co,(c,b)] += wcat-chunk^T @ xe  (PE, N=256, one PSUM bank;
            only diagonal c-blocks used) ; transpose + 1/Z fold ; squash
"""
import os
import numpy as np

C, B, R, CIN, OUT = 32, 64, 2048, 16, 32
NCORES = 8
CLOC = C // NCORES          # 4 capsules per core
RJ = R * CIN                # 32768
NK = RJ // 128              # 256 chunks of 128 along (j,r)
NV = RJ // 512              # 64 chunks of 512 along (r,j) for V pass

_CACHE = {}


def _build_program():
    from contextlib import ExitStack
    import concourse.bass as bass
    import concourse.bacc as bacc
    import concourse.tile as tile
    from concourse import mybir

    f32 = mybir.dt.float32
    f16 = mybir.dt.float16
    AL = mybir.AluOpType
    AF = mybir.ActivationFunctionType
    AX = mybir.AxisListType

    nc = bacc.Bacc(None, target_bir_lowering=False,
                   detect_race_conditions=not bool(int(os.environ.get("CAPS_NO_RACE", "0"))))
    n_loops = int(os.environ.get("CAPS_LOOPS", "1"))

    # ---- DRAM I/O (all host-swizzled to [partition, free] layouts) ----
    wcat = nc.dram_tensor("wcat", [128, NK * 128], f16, kind="ExternalInput")  # [p,(k,(c,o))]
    xt2 = nc.dram_tensor("xt2", [128, NK * B], f16, kind="ExternalInput")      # [p,(k,b)]
    wt = nc.dram_tensor("wt", [128, RJ], f16, kind="ExternalInput")            # [(c,o),(r,j)]
    x2d = nc.dram_tensor("x2d", [128, RJ], f16, kind="ExternalInput")          # [(2,b),(r,j)]
    ident = nc.dram_tensor("ident", [128, 128], f16, kind="ExternalInput")
    out3 = nc.dram_tensor("out3", [B, 128], f32, kind="ExternalOutput")        # [b,(c,o)]
    debug = bool(int(os.environ.get("CAPS_DEBUG", "0")))
    if debug:
        dbg_o1 = nc.dram_tensor("dbg_o1", [B, 128], f32, kind="ExternalOutput")
        dbg_lp = nc.dram_tensor("dbg_lp", [128, R], f32, kind="ExternalOutput")
        dbg_p2 = nc.dram_tensor("dbg_p2", [128, CLOC * (R // 128) * B], f16,
                                kind="ExternalOutput")
        dbg_s2 = nc.dram_tensor("dbg_s2", [B, 128], f32, kind="ExternalOutput")

    with tile.TileContext(nc) as tc, ExitStack() as ctx:
        const = ctx.enter_context(tc.tile_pool(name="const", bufs=1))
        small = ctx.enter_context(tc.tile_pool(name="small", bufs=2))
        wt_p = ctx.enter_context(tc.tile_pool(name="wtp", bufs=3))
        x2_p = ctx.enter_context(tc.tile_pool(name="x2p", bufs=2))
        vs_p = ctx.enter_context(tc.tile_pool(name="vsp", bufs=2))
        xe_p = ctx.enter_context(tc.tile_pool(name="xep", bufs=2))
        eP_p = ctx.enter_context(tc.tile_pool(name="ePp", bufs=2))
        tree_p = ctx.enter_context(tc.tile_pool(name="tree", bufs=1))
        psS_p = ctx.enter_context(tc.tile_pool(name="psS", bufs=1, space="PSUM"))
        psV_p = ctx.enter_context(tc.tile_pool(name="psV", bufs=4, space="PSUM"))
        psT_p = ctx.enter_context(tc.tile_pool(name="psT", bufs=2, space="PSUM"))

        idn = const.tile([128, 128], f16, tag="ident", name="idn")
        nc.sync.dma_start(out=idn, in_=ident[:])

        # resident W in [(j,r)-chunk, (c,o)] layout, fp16  (8.39 MB)
        wcat_sb = const.tile([128, NK, 128], f16, tag="wcat", name="wcat_sb")
        # resident x^T [(j,r)-chunk, b] fp16  (4.19 MB)
        xt2_sb = const.tile([128, NK, B], f16, tag="xt2sb", name="xt2_sb")
        NSPLIT = 16
        for i in range(NSPLIT):
            s0, s1 = NK // NSPLIT * i, NK // NSPLIT * (i + 1)
            nc.sync.dma_start(
                out=wcat_sb[:, s0:s1, :].rearrange("p k m -> p (k m)"),
                in_=wcat[:, 128 * s0:128 * s1])
            nc.sync.dma_start(
                out=xt2_sb[:, s0:s1, :].rearrange("p k b -> p (k b)"),
                in_=xt2[:, B * s0:B * s1])

        # logits per capsule-pair [(2c,b)=128, r=2048]
        # iter1 logits in fp16 (delta ~ +-10, err ~4e-3); iter2 in fp32
        lP16 = [const.tile([128, R], f16, tag=f"l16{p}", name=f"lP16{p}")
                for p in range(2)]
        # delta accumulator blocks (iteration 2)
        dblk = [const.tile([128, R], f32, tag=f"d{p}", name=f"dblk{p}") for p in range(2)]
        # transposed probs [128=r%128, (rb=16, c=4, b=64)] fp16 (flat for xe)
        p2T = const.tile([128, R // 128, CLOC, B], f16, tag="p2T", name="p2T")
        # blockdiag outT operand, capsule pair pr at partitions [64pr:64pr+64]
        oT2 = const.tile([128, 128], f16, tag="o2", name="oT2")
        nc.vector.memset(oT2[:], 0.0)

        def squash(u_bT, scale_pow, final=False):
            """u_bT [64,(4c,32o)] fp32: s=u*scale_pow; out=s*sqrt(n2)/(n2+1).

            Emits o_i (fp32 [B,128]) and fills oT2 blockdiag tiles (fp16)."""
            sq = small.tile([B, 128], f32, tag="sq", name="sq")
            nc.vector.scalar_tensor_tensor(
                out=sq, in0=u_bT, scalar=float(scale_pow * scale_pow),
                in1=u_bT, op0=AL.mult, op1=AL.mult)
            n2 = small.tile([B, CLOC], f32, tag="n2", name="n2")
            nc.vector.tensor_reduce(
                out=n2, in_=sq[:].rearrange("b (c o) -> b c o", c=CLOC),
                axis=AX.X, op=AL.add)
            rt = small.tile([B, CLOC], f32, tag="rt", name="rt")
            nc.scalar.activation(out=rt, in_=n2, func=AF.Sqrt)
            dn = small.tile([B, CLOC], f32, tag="dn", name="dn")
            nc.vector.tensor_scalar_add(out=dn, in0=n2, scalar1=1.0)
            rc = small.tile([B, CLOC], f32, tag="rc", name="rc")
            nc.vector.reciprocal(out=rc, in_=dn)
            f = small.tile([B, CLOC], f32, tag="f", name="f")
            nc.vector.tensor_mul(out=f, in0=rt, in1=rc)
            f2 = small.tile([B, CLOC], f32, tag="f2", name="f2")
            nc.vector.tensor_scalar_mul(out=f2, in0=f, scalar1=float(scale_pow))
            o_i = small.tile([B, 128], f32, tag="oi", name="oi")
            f2b = bass.AP(tensor=f2[:].tensor, offset=f2[:].offset,
                          ap=[f2[:].ap[0], f2[:].ap[1], [0, OUT]])
            nc.vector.tensor_tensor(
                out=o_i[:].rearrange("b (c o) -> b c o", c=CLOC),
                in0=u_bT[:].rearrange("b (c o) -> b c o", c=CLOC),
                in1=f2b, op=AL.mult)
            if final:
                return o_i
            o16 = small.tile([B, 128], f16, tag="o16", name="o16")
            nc.vector.tensor_copy(out=o16, in_=o_i)
            psOT = psT_p.tile([128, 128], f16, tag="psT", name="psOT")[:, 0:B]
            nc.tensor.transpose(psOT, o16, idn[0:B, 0:B])
            for p in range(2):
                nc.scalar.copy(out=oT2[64 * p:64 * p + 32, 0:64],
                               in_=psOT[64 * p:64 * p + 32, :])
                nc.scalar.copy(out=oT2[64 * p + 32:64 * p + 64, 64:128],
                               in_=psOT[64 * p + 32:64 * p + 64, :])
            return o_i

        xt2_flat = xt2_sb[:].rearrange("p k b -> p (k b)")
        p2T_flat = p2T[:].rearrange("p r c b -> p (r c b)")

        for _loop in range(n_loops):
            # ---------- Phase A: s1 = (1/R) sum_(j,r) x W ----------
            psA = psV_p.tile([128, 512], f32, tag="psV", name="psA")[:, 0:B]
            for k in range(NK):
                nc.tensor.matmul(psA, wcat_sb[:, k, :], xt2_sb[:, k, :],
                                 start=(k == 0), stop=(k == NK - 1))
            sA = small.tile([128, B], f16, tag="sA", name="sA")
            nc.scalar.copy(out=sA, in_=psA)
            psAT = psT_p.tile([128, 128], f16, tag="psT", name="psAT")[0:B, :]
            nc.tensor.transpose(psAT, sA, idn)
            uT = small.tile([B, 128], f32, tag="uT", name="uT")
            nc.scalar.copy(out=uT, in_=psAT)
            o1 = squash(uT, 1.0 / R)
            if debug:
                nc.sync.dma_start(out=dbg_o1[:], in_=o1)

            # ---------- Two routing boundaries ----------
            for it in (1, 2):
                # --- V pass: streamed wt/x2d quads of 2048 cols ---
                for g in range(16):
                    wtg = wt_p.tile([128, 2048], f16, tag="wtk", name="wtk")
                    nc.sync.dma_start(out=wtg, in_=wt[:, 2048 * g:2048 * (g + 1)])
                    x2g = x2_p.tile([128, 2048], f16, tag="x2k", name="x2k")
                    nc.sync.dma_start(out=x2g, in_=x2d[:, 2048 * g:2048 * (g + 1)])
                    for pr in range(2):
                        vs4 = vs_p.tile([128, 2048], f16, tag="vs", name="vs")
                        for q in range(4):
                            psV = psV_p.tile([128, 512], f32, tag="psV", name="psV")
                            nc.tensor.matmul(
                                psV, oT2[64 * pr:64 * (pr + 1), :],
                                wtg[64 * pr:64 * (pr + 1), 512 * q:512 * (q + 1)],
                                start=True, stop=True)
                            nc.scalar.copy(out=vs4[:, 512 * q:512 * (q + 1)],
                                           in_=psV)
                        xv4 = vs_p.tile([128, 2048], f16, tag="xv", name="xv")
                        nc.vector.tensor_mul(out=xv4, in0=x2g, in1=vs4)
                        dst = lP16[pr] if it == 1 else dblk[pr]
                        dsl = dst[:, 128 * g:128 * (g + 1)]
                        xv3 = xv4[:].rearrange("p (r j) -> p r j", j=CIN)
                        with nc.allow_low_precision(
                                reason="iter1 delta ~ +-10, fp16 err 4e-3 ok"):
                            if pr == 0 or g in (3, 7, 11):
                                # DVE grouped reduce
                                nc.vector.tensor_reduce(
                                    out=dsl, in_=xv3, axis=AX.X, op=AL.add)
                            else:
                                # GpSimd pairwise tree (fp32 intermediates)
                                t1 = tree_p.tile([128, 128, 8], f32, tag="t1",
                                                 name="t1")
                                nc.gpsimd.tensor_tensor(
                                    out=t1, in0=xv3[:, :, 0:8],
                                    in1=xv3[:, :, 8:16], op=AL.add)
                                t2 = tree_p.tile([128, 128, 4], f32, tag="t2",
                                                 name="t2")
                                nc.gpsimd.tensor_tensor(
                                    out=t2, in0=t1[:, :, 0:4], in1=t1[:, :, 4:8],
                                    op=AL.add)
                                t3 = tree_p.tile([128, 128, 2], f32, tag="t3",
                                                 name="t3")
                                nc.gpsimd.tensor_tensor(
                                    out=t3, in0=t2[:, :, 0:2], in1=t2[:, :, 2:4],
                                    op=AL.add)
                                nc.gpsimd.tensor_tensor(
                                    out=dsl.rearrange("p (r o) -> p r o", o=1),
                                    in0=t3[:, :, 0:1], in1=t3[:, :, 1:2],
                                    op=AL.add)
                if it == 2:
                    for pr in range(2):
                        nc.vector.tensor_add(out=dblk[pr], in0=lP16[pr],
                                             in1=dblk[pr])
                if debug and it == 1:
                    dbgl = small.tile([128, R], f32, tag="dbgl", name="dbgl")
                    nc.vector.tensor_copy(out=dbgl, in_=lP16[0])
                    nc.sync.dma_start(out=dbg_lp[:], in_=dbgl)

                # --- softmax pieces (unnormalized e + 1/Z) ---
                rzq = []
                for pr in range(2):
                    lsrc = lP16[pr] if it == 1 else dblk[pr]
                    m = small.tile([128, 1], f32, tag="m", name="m")
                    nc.vector.tensor_reduce(out=m, in_=lsrc, axis=AX.X, op=AL.max)
                    mneg = small.tile([128, 1], f32, tag="mneg", name="mneg")
                    nc.vector.tensor_scalar_mul(out=mneg, in0=m, scalar1=-1.0)
                    eP = eP_p.tile([128, R], f16, tag="eP", name="eP")
                    Z = small.tile([128, 1], f32, tag="Z", name="Z")
                    nc.scalar.activation(out=eP, in_=lsrc, func=AF.Exp,
                                         bias=mneg[:, 0:1], scale=1.0, accum_out=Z)
                    for ce in range(2):
                        rz = small.tile([B, 1], f32, tag=f"rz{2*pr+ce}", name="rz")
                        nc.vector.reciprocal(out=rz, in_=Z[64 * ce:64 * (ce + 1), 0:1])
                        rzq.append(rz)
                    for rb in range(R // 128):
                        psT2 = psT_p.tile([128, 128], f16, tag="psT", name="psT2")
                        nc.tensor.transpose(
                            psT2, eP[:, 128 * rb:128 * (rb + 1)], idn)
                        nc.vector.tensor_copy(
                            out=p2T[:, rb, 2 * pr:2 * pr + 2, :].rearrange(
                                "p e b -> p (e b)"),
                            in_=psT2)

                # --- xe + s matmuls: psS[co, (c,b)] = wcat^T @ xe4 ---
                # xe in [128, (t8, c4, b)] layout; only diagonal (c,c) psS
                # blocks are used. One matmul per 128-chunk (N=256).
                psS = psS_p.tile([128, CLOC * B], f32, tag="psS", name="psS")
                for j in range(CIN):
                    xe = xe_p.tile([128, 16, CLOC, B], f16, tag="xe",
                                   name="xe")
                    halves = ((0, 8), (8, 16)) if j <= 1 else ((0, 16),)
                    for (t0, t1) in halves:
                        nt = t1 - t0
                        c0 = B * (16 * j + t0)
                        x_sl = xt2_flat[:, c0:c0 + nt * B]
                        x_bc = bass.AP(tensor=x_sl.tensor, offset=x_sl.offset,
                                       ap=[x_sl.ap[0], [B, nt], [0, CLOC],
                                           [1, B]])
                        nc.vector.tensor_tensor(
                            out=xe[:, t0:t1, :, :], in0=x_bc,
                            in1=p2T[:, t0:t1, :, :],
                            op=AL.mult)
                        for t in range(t0, t1):
                            k = 16 * j + t
                            nc.tensor.matmul(
                                psS, wcat_sb[:, k, :],
                                xe[:, t, :, :],
                                start=(k == 0), stop=(k == NK - 1))
                sSt = small.tile([128, CLOC * B], f16, tag="sSt", name="sSt")
                nc.scalar.copy(out=sSt, in_=psS)
                sS = small.tile([B, 128], f32, tag="sS", name="sS")
                for c4 in range(CLOC):
                    psX = psT_p.tile([128, 128], f16, tag="psT",
                                     name="psX")[0:B, 0:32]
                    nc.tensor.transpose(
                        psX, sSt[32 * c4:32 * (c4 + 1), B * c4:B * (c4 + 1)],
                        idn[32 * c4:32 * c4 + 32, 32 * c4:32 * c4 + 32],
                        tile_position=(32 * c4, 0))
                    nc.scalar.activation(out=sS[:, 32 * c4:32 * (c4 + 1)],
                                         in_=psX,
                                         func=AF.Copy, bias=0.0,
                                         scale=rzq[c4][:, 0:1])
                if debug and it == 1:
                    nc.sync.dma_start(out=dbg_p2[:], in_=p2T_flat)
                    nc.sync.dma_start(out=dbg_s2[:], in_=sS)
                o_i = squash(sS, 1.0, final=(it == 2))

            nc.sync.dma_start(out=out3[:], in_=o_i)

    nc.finalize()
    return nc


def _get_program():
    if "nc" not in _CACHE:
        _CACHE["nc"] = _build_program()
    return _CACHE["nc"]


def make_in_maps(x, route_weights):
    x = np.ascontiguousarray(x, dtype=np.float32)
    W = np.ascontiguousarray(route_weights, dtype=np.float32)
    # x^T [(j,r), b] -> [p=128, (k, b)] fp16
    xt2f = x.transpose(2, 1, 0).reshape(NK, 128, B).transpose(1, 0, 2)
    xt2_h = np.ascontiguousarray(xt2f.reshape(128, NK * B), dtype=np.float16)
    # x natural [b, (r,j)] duplicated -> [128, RJ] fp16
    xnat = x.reshape(B, RJ)
    x2d_h = np.ascontiguousarray(
        np.concatenate([xnat, xnat], axis=0), dtype=np.float16)
    ident = np.eye(128, dtype=np.float16)
    in_maps = []
    for core in range(NCORES):
        wc = W[CLOC * core:CLOC * (core + 1)]                  # [4,R,J,O]
        # [(j,r), (c,o)] -> [p, (k, co)] fp16
        wcf = wc.transpose(2, 1, 0, 3).reshape(NK, 128, 128).transpose(1, 0, 2)
        wcat_h = np.ascontiguousarray(wcf.reshape(128, NK * 128), dtype=np.float16)
        # [(c,o), (r,j)] fp16
        wt_h = np.ascontiguousarray(
            wc.transpose(0, 3, 1, 2).reshape(128, RJ), dtype=np.float16)
        in_maps.append({"wcat": wcat_h, "xt2": xt2_h, "wt": wt_h,
                        "x2d": x2d_h, "ident": ident})
    return in_maps


def kernel(x, route_weights):
    from concourse.bass_utils import run_bass_kernel_spmd

    in_maps = make_in_maps(x, route_weights)
    nc = _get_program()
    res = run_bass_kernel_spmd(nc, in_maps, core_ids=list(range(NCORES)))
    if os.environ.get("CAPS_RESULT_STASH"):
        _CACHE["last_result"] = res

    out = np.empty((C, B, 1, 1, OUT), dtype=np.float32)
    for core in range(NCORES):
        o = res.results[core]["out3"].reshape(B, CLOC, OUT).transpose(1, 0, 2)
        out[CLOC * core:CLOC * (core + 1), :, 0, 0, :] = o
    return out
